# revision 1
# baseline (speedup 1.0000x reference)
"""LIF (leaky integrate-and-fire) spiking recurrence on 8 Trainium2 cores.

Full input x: [T*bs, C, H, W] = [256, 128, 32, 32] f32 with T=8, bs=32.
Recurrence over T only, elementwise elsewhere:
    u_t = TAU * u_{t-1} * (1 - (u_{t-1} > VTH)) + x_t ;  o_t = (u_t > VTH)

Sharding: fully data-parallel over batch (bs=32 -> 4 per core), no collectives.

Since the op is elementwise outside of T, each core views its [4,128,32,32]
per-timestep slab as a flat [128 partitions, 4096] tile (16 KiB contiguous
HBM run per partition -> large DMA descriptors). Each timestep is split into
CH chunks so compute and output stores start as early as possible; the two
chunk chains interleave on DVE and hide cross-engine stalls.

Per step and chunk:
  DVE : u = (p * TAU) + x_t            (scalar_tensor_tensor; t=0: u=x_0)
  ACT : s = sign(VTH - u); o = relu(-s) = (u > VTH)   (exact: u-VTH exact near VTH)
  DVE : p = (u <= VTH) * u             (skipped at t=T-1)
TAU=0.5 is a power of two and the masks are 0/1, so everything except the
add is exact -> bitwise identical to the f32 reference.
"""

import numpy as np

import concourse.tile as tile
from concourse import bacc, mybir
from concourse.bass_utils import run_bass_kernel_spmd

T = 8
BS = 32
C = 128
HW = 32 * 32
NCORES = 8
BSH = BS // NCORES          # 4 batch elements per core
P = 128                     # SBUF partitions
FREE = BSH * C * HW // P    # 4096 f32 per partition per timestep
CH = 2                      # chunks per timestep
CHF = FREE // CH            # 2048
VTH = 1.0
TAU = 0.5
F32 = mybir.dt.float32

_nc_cache = None


def _build():
    nc = bacc.Bacc("TRN2", target_bir_lowering=False, debug=False, num_devices=NCORES)
    x_d = nc.dram_tensor("x", [T, P, FREE], F32, kind="ExternalInput").ap()
    o_d = nc.dram_tensor("o", [T, P, FREE], F32, kind="ExternalOutput").ap()

    BF16 = mybir.dt.bfloat16

    with tile.TileContext(nc) as tc:
        with (
            tc.tile_pool(name="xa", bufs=1) as xa,
            tc.tile_pool(name="pp", bufs=1) as pp,
            tc.tile_pool(name="sp", bufs=2) as sp,
            tc.tile_pool(name="op", bufs=2) as op,
        ):
            # The whole 16 MiB per-core input stays resident in SBUF
            # (128 KiB/partition); the u state is computed in place over it,
            # so no separate u pool is needed. Subtile dependency tracking
            # lets each compute chunk start as soon as the load covering its
            # range lands.
            xt = xa.tile([P, T * FREE], F32)
            xv = x_d.rearrange("t p f -> p t f")  # [128, T, FREE] HBM view

            # Ramped load sizes (units of CHF=2048 halves): small first so
            # compute starts early, large later so the SP ring issues only a
            # few DMAs and the full input is resident early.
            load_ranges = [(0, 1), (1, 2), (2, 4), (4, 6), (6, 8), (8, 12), (12, 16)]
            for a, b in load_ranges:
                t0, f0 = divmod(a * CHF, FREE)
                t1, f1 = divmod(b * CHF, FREE)
                if f0 == 0 and f1 == 0:
                    src = xv[:, t0:t1, :]
                else:
                    assert t1 == t0 and f1 > f0 or (t1 == t0 + 1 and f1 == 0)
                    src = xv[:, t0, f0:f1 if f1 else FREE]
                nc.sync.dma_start(out=xt[:, a * CHF:b * CHF], in_=src)

            p_prev = None
            for t in range(T):
                # Full-timestep ops in the steady state (fewest instructions
                # and semaphores); halves at t=0 for an early pipeline start
                # and at t=T-1 for a short latency tail.
                nh = 2 if t in (0, T - 1) else 1
                w = FREE // nh
                s = sp.tile([P, FREE], BF16, name="s", tag="s")
                o = op.tile([P, FREE], F32, name="o", tag="o")
                pn = (
                    pp.tile([P, FREE], F32, name="p", tag="p")
                    if t < T - 1 else None
                )
                for c in range(nh):
                    fsl = slice(c * w, (c + 1) * w)
                    xsl = xt[:, t * FREE + c * w:t * FREE + (c + 1) * w]
                    if t > 0:
                        # u = p*TAU + x, in place over the x slice
                        nc.vector.scalar_tensor_tensor(
                            xsl, p_prev[:, fsl], TAU, xsl,
                            op0=mybir.AluOpType.mult, op1=mybir.AluOpType.add,
                        )
                    # s = sign(VTH - u) in bf16 (exact on {-1,0,1}), then
                    # o = relu(-s) = (u > VTH) in f32 for the store. Signs
                    # are flipped via the scale immediate because only
                    # 0.0/1.0 have pre-registered const APs for the bias.
                    nc.scalar.activation(
                        s[:, fsl], xsl, mybir.ActivationFunctionType.Sign,
                        bias=VTH, scale=-1.0,
                    )
                    nc.scalar.activation(
                        o[:, fsl], s[:, fsl],
                        mybir.ActivationFunctionType.Relu, scale=-1.0,
                    )
                    if pn is not None:
                        nc.vector.scalar_tensor_tensor(
                            pn[:, fsl], xsl, VTH, xsl,
                            op0=mybir.AluOpType.is_le, op1=mybir.AluOpType.mult,
                        )
                    # Stores go out on the GpSimd SWDGE ring: its queue rows
                    # are separate from the SP HWDGE ring, so stores are not
                    # FIFO-blocked behind the big prefetch loads. The last
                    # timestep's stores are quartered for a short tail.
                    if t == T - 1:
                        q = w // 2
                        nc.gpsimd.dma_start(
                            out=o_d[t][:, c * w:c * w + q],
                            in_=o[:, c * w:c * w + q],
                        )
                        nc.gpsimd.dma_start(
                            out=o_d[t][:, c * w + q:(c + 1) * w],
                            in_=o[:, c * w + q:(c + 1) * w],
                        )
                    else:
                        nc.gpsimd.dma_start(out=o_d[t][:, fsl], in_=o[:, fsl])
                p_prev = pn

    nc.compile()
    return nc


def _get_nc():
    global _nc_cache
    if _nc_cache is None:
        _nc_cache = _build()
    return _nc_cache


def _run(x: np.ndarray, **spmd_kwargs):
    nc = _get_nc()
    xr = np.ascontiguousarray(np.asarray(x, dtype=np.float32)).reshape(T, BS, C, HW)
    in_maps = [
        {"x": np.ascontiguousarray(xr[:, k * BSH:(k + 1) * BSH]).reshape(T, P, FREE)}
        for k in range(NCORES)
    ]
    res = run_bass_kernel_spmd(nc, in_maps, core_ids=list(range(NCORES)), **spmd_kwargs)
    out = np.empty((T, BS, C, HW), dtype=np.float32)
    for k in range(NCORES):
        out[:, k * BSH:(k + 1) * BSH] = res.results[k]["o"].reshape(T, BSH, C, HW)
    return out.reshape(T * BS, C, 32, 32), res


def kernel(x: np.ndarray) -> np.ndarray:
    out, _ = _run(x)
    return out



# revision 4
# speedup vs baseline: 1.1108x; 1.1108x over previous
"""LIF (leaky integrate-and-fire) spiking recurrence on 8 Trainium2 cores.

Full input x: [T*bs, C, H, W] = [256, 128, 32, 32] f32 with T=8, bs=32.
Recurrence over T only, elementwise elsewhere:
    u_t = TAU * u_{t-1} * (1 - (u_{t-1} > VTH)) + x_t ;  o_t = (u_t > VTH)

Sharding: fully data-parallel over batch (bs=32 -> 4 per core), no collectives.

Since the op is elementwise outside of T, each core views its [4,128,32,32]
per-timestep slab as a flat [128 partitions, 4096] tile (16 KiB contiguous
HBM run per partition -> large DMA descriptors).

The output is pure 0/1 spikes, so it is stored as ONE BYTE per element:
ACT computes s = sign(u - VTH) in {-1,0,+1} directly as int8 (exact; no
reliance on saturation), and the host maps byte==1 -> 1.0f. This cuts the
store traffic 4x vs f32 and takes the kernel from ~97us (f32 roofline)
toward the ~60us roofline of 16.78MB in + 4.19MB out per core.

Per step:
  DVE : u = (p * TAU) + x_t        (scalar_tensor_tensor; t=0: u=x_0)
  ACT : s = sign(u - VTH) -> int8  (single pass; spike iff byte == 1)
  DVE : p = (u <= VTH) * u         (skipped at t=T-1)
TAU=0.5 is a power of two and the masks are 0/1, so everything except the
add is exact -> bitwise identical to the f32 reference.
"""

import numpy as np

import concourse.tile as tile
from concourse import bacc, mybir
from concourse.bass_utils import run_bass_kernel_spmd

T = 8
BS = 32
C = 128
HW = 32 * 32
NCORES = 8
BSH = BS // NCORES          # 4 batch elements per core
P = 128                     # SBUF partitions
FREE = BSH * C * HW // P    # 4096 f32 per partition per timestep
CH = 2                      # chunks per timestep
CHF = FREE // CH            # 2048
VTH = 1.0
TAU = 0.5
F32 = mybir.dt.float32
I8 = mybir.dt.int8

_nc_cache = None


def _build():
    nc = bacc.Bacc("TRN2", target_bir_lowering=False, debug=False, num_devices=NCORES)
    x_d = nc.dram_tensor("x", [T, P, FREE], F32, kind="ExternalInput").ap()
    o_d = nc.dram_tensor("o", [T, P, FREE], I8, kind="ExternalOutput").ap()

    with tile.TileContext(nc) as tc:
        with (
            tc.tile_pool(name="xa", bufs=1) as xa,
            tc.tile_pool(name="pp", bufs=1) as pp,
            tc.tile_pool(name="op", bufs=2) as op,
            tc.tile_pool(name="cp", bufs=1) as cp,
        ):
            # Per-partition bias constant -VTH for the ACT sign pass (only
            # 0.0/1.0 have pre-registered const APs).
            nbias = cp.tile([P, 1], F32)
            nc.vector.memset(nbias[:, :], -VTH)

            # The whole 16 MiB per-core input stays resident in SBUF
            # (128 KiB/partition); the u state is computed in place over it.
            xt = xa.tile([P, T * FREE], F32)
            xv = x_d.rearrange("t p f -> p t f")  # [128, T, FREE] HBM view

            # Ramped load sizes (units of CHF=2048 halves): small first so
            # compute starts early, large later so the SP ring issues only a
            # few DMAs and the full input is resident early.
            load_ranges = [(0, 1), (1, 2), (2, 4), (4, 6), (6, 8), (8, 12), (12, 16)]
            for a, b in load_ranges:
                t0, f0 = divmod(a * CHF, FREE)
                t1, f1 = divmod(b * CHF, FREE)
                if f0 == 0 and f1 == 0:
                    src = xv[:, t0:t1, :]
                else:
                    assert t1 == t0 and f1 > f0 or (t1 == t0 + 1 and f1 == 0)
                    src = xv[:, t0, f0:f1 if f1 else FREE]
                nc.sync.dma_start(out=xt[:, a * CHF:b * CHF], in_=src)

            p_prev = None
            for t in range(T):
                # Full-timestep ops in the steady state (fewest instructions
                # and semaphores); halves at t=0 for an early pipeline start
                # and at t=T-1 for a short latency tail.
                nh = 2 if t in (0, T - 1) else 1
                w = FREE // nh
                o = op.tile([P, FREE], I8, name="o", tag="o")
                pn = (
                    pp.tile([P, FREE], F32, name="p", tag="p")
                    if t < T - 1 else None
                )
                for c in range(nh):
                    fsl = slice(c * w, (c + 1) * w)
                    xsl = xt[:, t * FREE + c * w:t * FREE + (c + 1) * w]
                    if t > 0:
                        # u = p*TAU + x, in place over the x slice
                        nc.vector.scalar_tensor_tensor(
                            xsl, p_prev[:, fsl], TAU, xsl,
                            op0=mybir.AluOpType.mult, op1=mybir.AluOpType.add,
                        )
                    # s = sign(u - VTH) in int8: +1 iff spike, {-1,0} iff not.
                    # Exact regardless of int conversion saturation semantics.
                    nc.scalar.activation(
                        o[:, fsl], xsl, mybir.ActivationFunctionType.Sign,
                        bias=nbias[:, :], scale=1.0,
                    )
                    if pn is not None:
                        nc.vector.scalar_tensor_tensor(
                            pn[:, fsl], xsl, VTH, xsl,
                            op0=mybir.AluOpType.is_le, op1=mybir.AluOpType.mult,
                        )
                    # Stores go out on the GpSimd SWDGE ring: its queue rows
                    # are separate from the SP HWDGE ring, so stores are not
                    # FIFO-blocked behind the big prefetch loads.
                    if t == T - 1:
                        q = w // 2
                        nc.gpsimd.dma_start(
                            out=o_d[t][:, c * w:c * w + q],
                            in_=o[:, c * w:c * w + q],
                        )
                        nc.gpsimd.dma_start(
                            out=o_d[t][:, c * w + q:(c + 1) * w],
                            in_=o[:, c * w + q:(c + 1) * w],
                        )
                    else:
                        nc.gpsimd.dma_start(out=o_d[t][:, fsl], in_=o[:, fsl])
                p_prev = pn

    nc.compile()
    return nc


def _get_nc():
    global _nc_cache
    if _nc_cache is None:
        _nc_cache = _build()
    return _nc_cache


def _run(x: np.ndarray, **spmd_kwargs):
    nc = _get_nc()
    xr = np.ascontiguousarray(np.asarray(x, dtype=np.float32)).reshape(T, BS, C, HW)
    in_maps = [
        {"x": np.ascontiguousarray(xr[:, k * BSH:(k + 1) * BSH]).reshape(T, P, FREE)}
        for k in range(NCORES)
    ]
    res = run_bass_kernel_spmd(nc, in_maps, core_ids=list(range(NCORES)), **spmd_kwargs)
    out = np.empty((T, BS, C, HW), dtype=np.float32)
    for k in range(NCORES):
        ok = res.results[k]["o"].reshape(T, BSH, C, HW)
        out[:, k * BSH:(k + 1) * BSH] = (ok == 1)
    return out.reshape(T * BS, C, 32, 32), res


def kernel(x: np.ndarray) -> np.ndarray:
    out, _ = _run(x)
    return out
